# revision 1
# baseline (speedup 1.0000x reference)
import os
import sys

for _p in ("/opt/trn_rl_repo",):
    if os.path.isdir(_p) and _p not in sys.path:
        sys.path.insert(0, _p)

import numpy as np
import ml_dtypes

import concourse.bass as bass
import concourse.tile as tile
from concourse import bacc
from concourse import mybir
from concourse import bass_utils
from concourse.alu_op_type import AluOpType

BF16 = ml_dtypes.bfloat16
AF = mybir.ActivationFunctionType

S = 1560
DIM = 1536
NH = 12
HD = 128
CACHE = 4680
NCORES = 8
RPC = S // NCORES
EPS = 1e-6
LOCAL_ATTN_SIZE = 3
SINK_SIZE = 1
MAX_ATTN = 32760 if LOCAL_ATTN_SIZE == -1 else LOCAL_ATTN_SIZE * S

NKC = (CACHE + 127) // 128
TAIL = CACHE - (NKC - 1) * 128

RCHUNKS = [(0, 128), (128, 195)]

_CACHED = {}
LAST_RUNS = []


def _build_launch1():
    nc = bacc.Bacc("TRN2", target_bir_lowering=False, debug=False,
                   num_devices=NCORES, num_swdge_queues=4)
    f32, bf = mybir.dt.float32, mybir.dt.bfloat16

    xt_d = nc.dram_tensor("xt", [128, 12, RPC], bf, kind="ExternalInput")
    w3_d = nc.dram_tensor("w3", [9, 128, 12, 512], bf, kind="ExternalInput")
    cq_d = nc.dram_tensor("cq", [RPC, DIM], bf, kind="ExternalInput")
    sq_d = nc.dram_tensor("sq", [RPC, DIM], bf, kind="ExternalInput")
    ck_d = nc.dram_tensor("ck", [RPC, DIM], bf, kind="ExternalInput")
    sk_d = nc.dram_tensor("sk", [RPC, DIM], bf, kind="ExternalInput")
    out_d = nc.dram_tensor("qkv", [RPC, 3 * DIM], bf, kind="ExternalOutput")

    with tile.TileContext(nc) as tc:
        with (
            tc.tile_pool(name="consts", bufs=1) as consts,
            tc.tile_pool(name="wstream", bufs=3) as wstream,
            tc.tile_pool(name="stage", bufs=1) as stagep,
            tc.tile_pool(name="ps", bufs=4, space="PSUM") as psp,
            tc.tile_pool(name="small", bufs=2) as small,
            tc.tile_pool(name="outs", bufs=1) as outsp,
            tc.tile_pool(name="tmp", bufs=1) as tmpp,
        ):
            xt = consts.tile([128, 12, RPC], bf)
            nc.sync.dma_start(xt[:], xt_d.ap())

            stage = {}
            for ti in range(2):
                for ri, (r0, r1) in enumerate(RCHUNKS):
                    stage[(ti, ri)] = stagep.tile([r1 - r0, DIM], f32,
                                                  tag=f"st{ti}{ri}", name=f"st{ti}{ri}")

            ssq = {}
            for ti in range(2):
                for ri, (r0, r1) in enumerate(RCHUNKS):
                    for ns in range(3):
                        ssq[(ti, ri, ns)] = small.tile(
                            [r1 - r0, 1], f32, tag=f"ssq{ti}{ri}{ns}", name=f"ssq{ti}{ri}{ns}")

            outt = {ri: outsp.tile([r1 - r0, 3 * DIM], bf, tag=f"out{ri}", name=f"out{ri}")
                    for ri, (r0, r1) in enumerate(RCHUNKS)}

            sq_scratch = {ri: tmpp.tile([r1 - r0, 512], bf, tag=f"sqs{ri}", name=f"sqs{ri}")
                          for ri, (r0, r1) in enumerate(RCHUNKS)}

            epsb = consts.tile([128, 1], f32, name="epsb")
            nc.vector.memset(epsb[:], EPS)

            tabs = {}
            tab_specs = [(name, dram, ri)
                         for name, dram in (("cq", cq_d), ("sq", sq_d),
                                            ("ck", ck_d), ("sk", sk_d))
                         for ri in range(len(RCHUNKS))]
            for n in range(9):
                wt = wstream.tile([128, 12, 512], bf, tag="w", name="wt")
                eng = (nc.sync, nc.scalar, nc.gpsimd)[n % 3]
                eng.dma_start(wt[:], w3_d.ap()[n])
                if n >= 1 and tab_specs:
                    name, dram, ri = tab_specs.pop(0)
                    r0, r1 = RCHUNKS[ri]
                    t = consts.tile([r1 - r0, DIM], bf,
                                    tag=f"tab{name}{ri}",
                                    name=f"tab{name}{ri}")
                    (nc.scalar if n % 2 else nc.gpsimd).dma_start(
                        t[:], dram.ap()[r0:r1, :])
                    tabs[(name, ri)] = t
                ti, ns = divmod(n, 3)
                for ri, (r0, r1) in enumerate(RCHUNKS):
                    rs = r1 - r0
                    pr = psp.tile([128, 512], f32, tag="pr", name="pr")
                    for kc in range(12):
                        nc.tensor.matmul(
                            pr[:rs, :],
                            xt[:, kc, r0:r1],
                            wt[:, kc, :],
                            start=(kc == 0),
                            stop=(kc == 11),
                        )
                    if ti < 2:
                        nc.scalar.activation(
                            out=sq_scratch[ri][:rs, :],
                            in_=pr[:rs, :],
                            func=AF.Square,
                            accum_out=ssq[(ti, ri, ns)][:rs, :],
                        )
                        nc.vector.tensor_copy(
                            stage[(ti, ri)][:rs, ns * 512:(ns + 1) * 512],
                            pr[:rs, :],
                        )
                    else:
                        nc.vector.tensor_copy(
                            outt[ri][:rs, 2 * DIM + ns * 512:
                                     2 * DIM + (ns + 1) * 512],
                            pr[:rs, :],
                        )

            for ti, (cn, sn) in ((0, ("cq", "sq")), (1, ("ck", "sk"))):
                for ri, (r0, r1) in enumerate(RCHUNKS):
                    rs = r1 - r0
                    st = stage[(ti, ri)]
                    tot = small.tile([rs, 1], f32, tag=f"tot{ti}{ri}", name=f"tot{ti}{ri}")
                    nc.vector.tensor_tensor(
                        tot[:], ssq[(ti, ri, 0)][:rs, :],
                        ssq[(ti, ri, 1)][:rs, :], AluOpType.add)
                    nc.vector.tensor_tensor(
                        tot[:], tot[:], ssq[(ti, ri, 2)][:rs, :],
                        AluOpType.add)
                    nc.scalar.activation(out=tot[:], in_=tot[:], func=AF.Sqrt,
                                         bias=epsb[:rs, :], scale=1.0 / DIM)
                    nc.vector.reciprocal(out=tot[:], in_=tot[:])
                    if ti == 0:
                        nc.vector.tensor_scalar_mul(
                            tot[:], tot[:], 1.0 / float(np.sqrt(HD)))

                    sw = tmpp.tile([rs, DIM], f32, tag=f"sw{ri}", name=f"sw{ri}")
                    st3 = st[:rs, :].rearrange("p (c two) -> p c two", two=2)
                    sw3 = sw[:rs, :].rearrange("p (c two) -> p c two", two=2)
                    nc.scalar.copy(sw3[:, :, 0], st3[:, :, 1])
                    nc.scalar.copy(sw3[:, :, 1], st3[:, :, 0])
                    t1 = tmpp.tile([rs, DIM], f32, tag=f"t1{ri}", name=f"t1{ri}")
                    nc.vector.tensor_tensor(
                        t1[:], st[:rs, :], tabs[(cn, ri)][:], AluOpType.mult)
                    nc.vector.tensor_tensor(
                        sw[:rs, :], sw[:rs, :], tabs[(sn, ri)][:],
                        AluOpType.mult)
                    nc.vector.tensor_tensor(
                        t1[:], t1[:], sw[:rs, :], AluOpType.add)
                    nc.scalar.activation(
                        out=outt[ri][:rs, ti * DIM:(ti + 1) * DIM],
                        in_=t1[:], func=AF.Copy, scale=tot[:])

            for ri, (r0, r1) in enumerate(RCHUNKS):
                for s, eng in ((0, nc.sync), (1, nc.scalar), (2, nc.gpsimd)):
                    eng.dma_start(
                        out_d.ap()[r0:r1, s * DIM:(s + 1) * DIM],
                        outt[ri][:, s * DIM:(s + 1) * DIM])

    nc.finalize()
    return nc


def _build_launch2():
    nc = bacc.Bacc("TRN2", target_bir_lowering=False, debug=False,
                   num_devices=NCORES, num_swdge_queues=4)
    f32, bf = mybir.dt.float32, mybir.dt.bfloat16

    qt_d = nc.dram_tensor("qt", [128, 12, RPC], bf, kind="ExternalInput")
    kt_d = nc.dram_tensor("kt", [NH, 128, CACHE], bf, kind="ExternalInput")
    vt_d = nc.dram_tensor("vt", [NH, 128, NKC, 128], bf, kind="ExternalInput")
    w2_d = nc.dram_tensor("w2", [128, 12, 3, 512], bf, kind="ExternalInput")
    bo_d = nc.dram_tensor("bo", [1, DIM], f32, kind="ExternalInput")
    out_d = nc.dram_tensor("outp", [RPC, DIM], f32, kind="ExternalOutput")

    with tile.TileContext(nc) as tc:
        with (
            tc.tile_pool(name="consts", bufs=1) as consts,
            tc.tile_pool(name="kv", bufs=2) as kvp,
            tc.tile_pool(name="p", bufs=8) as pp,
            tc.tile_pool(name="acc", bufs=1) as accp,
            tc.tile_pool(name="lp", bufs=3, space="PSUM") as lpp,
            tc.tile_pool(name="ops", bufs=1, space="PSUM") as opsp,
            tc.tile_pool(name="pop", bufs=1, space="PSUM") as popp,
            tc.tile_pool(name="small", bufs=4) as small,
            tc.tile_pool(name="outs", bufs=2) as outsp,
        ):
            qt = consts.tile([128, 12, RPC], bf)
            nc.sync.dma_start(qt[:], qt_d.ap())
            w2 = consts.tile([128, 12, 3, 512], bf)
            nc.sync.dma_start(w2[:], w2_d.ap())
            bo_b = consts.tile([128, DIM], f32)
            nc.sync.dma_start(
                bo_b[:],
                bass.AP(tensor=bo_d, offset=0, ap=[[0, 128], [1, DIM]]))
            ones = consts.tile([128, 1], f32)
            nc.vector.memset(ones[:], 1.0)
            ones_row = consts.tile([1, 128], f32)
            nc.vector.memset(ones_row[:], 1.0)
            o3 = consts.tile([128, 12, RPC], bf)
            wsrc = consts.tile([128, 512], bf, name="wsrc")
            nc.vector.memset(wsrc[:], 0.0)
            for wu in range(24):
                wp = lpp.tile([128, 1024], f32, tag="lp", name="lpw")
                nc.tensor.matmul(wp[:, 0:512], wsrc[:, :128], wsrc[:],
                                 start=True, stop=True)
            o3u = consts.tile([128, 12, RPC], f32)
            saccs = []

            def denom_chain(h):
                sab = saccs[h]
                nc.vector.tensor_tensor(sab[0][:], sab[0][:], sab[1][:],
                                        AluOpType.add)
                nc.vector.tensor_tensor(
                    sab[0][:, 0:196], sab[0][:, 0:196], sab[0][:, 196:392],
                    AluOpType.add)
                srow = lpp.tile([1, RPC], f32, tag="lp", name="srow")
                nc.tensor.matmul(srow[:], ones[:], sab[0][:, 0:RPC],
                                 start=True, stop=True)
                sinv = small.tile([1, RPC], f32, tag="sinv", name="sinv")
                nc.vector.reciprocal(out=sinv[:], in_=srow[:])
                sinv_p = opsp.tile([128, RPC], f32, tag="opsum",
                                   name="sinvp")
                nc.tensor.matmul(sinv_p[:], ones_row[:], sinv[:],
                                 start=True, stop=True)
                nc.vector.tensor_tensor(
                    o3[:, h, :], o3u[:, h, :], sinv_p[:], AluOpType.mult)

            for h in range(NH):
                kt = kvp.tile([128, CACHE], bf, tag="kt", name="ktile")
                vt = kvp.tile([128, NKC, 128], bf, tag="vt", name="vtile")
                if h == 0:
                    half = 2304
                    nc.sync.dma_start(kt[:, :half], kt_d.ap()[h][:, :half])
                    nc.sync.dma_start(kt[:, half:], kt_d.ap()[h][:, half:])
                    nc.gpsimd.dma_start(vt[:, :18, :], vt_d.ap()[h][:, :18, :])
                    nc.gpsimd.dma_start(vt[:, 18:, :], vt_d.ap()[h][:, 18:, :])
                else:
                    nc.sync.dma_start(kt[:], kt_d.ap()[h])
                    nc.gpsimd.dma_start(vt[:], vt_d.ap()[h])

                opsum = opsp.tile([128, RPC], f32, tag="opsum", name="opsum")
                sacc_ab = [accp.tile([128, 392], f32, tag=f"sacc{h}{ab}",
                                     name=f"sacc{h}{ab}") for ab in range(2)]
                for sa in sacc_ab:
                    nc.vector.memset(sa[:], 0.0)

                OFFS = (0, 196, 512, 708)
                for jj in range(0, NKC - 1, 4):
                    lp = lpp.tile([128, 1024], f32, tag="lp", name="lp")
                    pt = pp.tile([128, 1024], bf, tag="pt", name="pt")
                    for u in range(4):
                        j = jj + u
                        o0 = OFFS[u]
                        nc.tensor.matmul(
                            lp[:, o0:o0 + RPC],
                            kt[:, j * 128:(j + 1) * 128],
                            qt[:, h, :],
                            start=True, stop=True)
                    nc.scalar.activation(out=pt[:], in_=lp[:], func=AF.Exp)
                    for u in range(4):
                        j = jj + u
                        o0 = OFFS[u]
                        nc.tensor.matmul(
                            opsum[:],
                            vt[:, j, :],
                            pt[:, o0:o0 + RPC],
                            start=(j == 0), stop=False)
                    padd = pp.tile([128, 392], bf, tag="padd", name="padd")
                    nc.vector.tensor_tensor(
                        padd[:], pt[:, 0:392], pt[:, 512:904],
                        AluOpType.add)
                    sa = sacc_ab[(jj // 4) % 2]
                    nc.vector.tensor_tensor(sa[:], sa[:], padd[:],
                                            AluOpType.add)

                j = NKC - 1
                lp = lpp.tile([128, 1024], f32, tag="lp", name="lp")
                pt = pp.tile([128, 1024], bf, tag="pt", name="pt")
                nc.tensor.matmul(
                    lp[:TAIL, :RPC],
                    kt[:, j * 128:j * 128 + TAIL],
                    qt[:, h, :],
                    start=True, stop=True)
                nc.scalar.activation(out=pt[:TAIL, :RPC],
                                     in_=lp[:TAIL, :RPC], func=AF.Exp)
                nc.tensor.matmul(
                    opsum[:], vt[:TAIL, j, :], pt[:TAIL, :RPC],
                    start=False, stop=True)
                nc.vector.tensor_tensor(
                    sacc_ab[0][:TAIL, 0:RPC], sacc_ab[0][:TAIL, 0:RPC],
                    pt[:TAIL, :RPC], AluOpType.add)

                nc.vector.tensor_copy(o3u[:, h, :], opsum[:])
                saccs.append(sacc_ab)

            for h in range(NH):
                denom_chain(h)

            outf = {ri: outsp.tile([r1 - r0, DIM], f32, tag=f"of{ri}", name=f"of{ri}")
                    for ri, (r0, r1) in enumerate(RCHUNKS)}
            for ri, (r0, r1) in enumerate(RCHUNKS):
                rs = r1 - r0
                for nf in range(3):
                    po = popp.tile([128, 512], f32, tag="po", name="po")
                    for h in range(NH):
                        nc.tensor.matmul(
                            po[:rs, :],
                            o3[:, h, r0:r1],
                            w2[:, h, nf, :],
                            start=(h == 0), stop=(h == NH - 1))
                    nc.vector.tensor_tensor(
                        outf[ri][:rs, nf * 512:(nf + 1) * 512],
                        po[:rs, :],
                        bo_b[:rs, nf * 512:(nf + 1) * 512],
                        AluOpType.add)
                    eng = (nc.sync, nc.scalar, nc.gpsimd)[nf]
                    eng.dma_start(
                        out_d.ap()[r0:r1, nf * 512:(nf + 1) * 512],
                        outf[ri][:rs, nf * 512:(nf + 1) * 512])

    nc.finalize()
    return nc


def _cache_plan(current_start, global_end_index, local_end_index, s, kv_size,
                frame_seqlen):
    current_end = current_start + s
    sink_tokens = SINK_SIZE * frame_seqlen

    kind = np.zeros(kv_size, dtype=np.int64)
    idx = np.arange(kv_size, dtype=np.int64)

    if (LOCAL_ATTN_SIZE != -1 and current_end > global_end_index
            and s + local_end_index > kv_size):
        num_evicted = s + local_end_index - kv_size
        num_rolled = local_end_index - num_evicted - sink_tokens
        src0 = sink_tokens + num_evicted
        kind[sink_tokens:sink_tokens + num_rolled] = \
            kind[src0:src0 + num_rolled]
        idx[sink_tokens:sink_tokens + num_rolled] = \
            idx[src0:src0 + num_rolled]
        new_local_end = (local_end_index + current_end - global_end_index
                         - num_evicted)
    else:
        new_local_end = local_end_index + current_end - global_end_index
    local_start = new_local_end - s
    is_recompute = (current_end <= global_end_index) and (current_start > 0)
    write_start = max(local_start, sink_tokens) if is_recompute \
        else local_start
    off = max(0, write_start - local_start)
    wl = max(0, new_local_end - write_start)
    if wl > 0:
        kind[write_start:new_local_end] = 1
        idx[write_start:new_local_end] = off + np.arange(wl)

    if sink_tokens > 0:
        budget = MAX_ATTN - sink_tokens
        if budget > 0:
            lo = max(sink_tokens, new_local_end - budget)
            sel = np.concatenate([np.arange(sink_tokens),
                                  np.arange(lo, new_local_end)])
        else:
            sel = np.arange(sink_tokens)
    else:
        ws = max(0, new_local_end - MAX_ATTN)
        sel = np.arange(ws, new_local_end)

    k_kind, k_idx = kind[sel], idx[sel]
    old_rows = k_idx[k_kind == 0]
    new_rows = k_idx[k_kind == 1]
    return old_rows, new_rows


def _rope_tables(freqs_real, freqs_imag, f, h, w, start_frame, gq, gk):
    c = HD // 2
    c0 = c - 2 * (c // 3)
    c1 = c // 3
    fr = np.asarray(freqs_real, np.float32)
    fi = np.asarray(freqs_imag, np.float32)
    s = f * h * w
    assert s == S
    fidx = np.arange(s) // (h * w)
    hidx = (np.arange(s) // w) % h
    widx = np.arange(s) % w
    fr_pos = np.concatenate([
        fr[start_frame + fidx][:, :c0],
        fr[hidx][:, c0:c0 + c1],
        fr[widx][:, c0 + c1:c0 + 2 * c1],
    ], axis=1)
    fi_pos = np.concatenate([
        fi[start_frame + fidx][:, :c0],
        fi[hidx][:, c0:c0 + c1],
        fi[widx][:, c0 + c1:c0 + 2 * c1],
    ], axis=1)
    C1 = np.repeat(fr_pos, 2, axis=1)
    Sg = np.empty((s, HD), np.float32)
    Sg[:, 0::2] = -fi_pos
    Sg[:, 1::2] = fi_pos
    C = np.tile(C1, (1, NH))
    Sx = np.tile(Sg, (1, NH))
    gq = np.asarray(gq, np.float32)
    gk = np.asarray(gk, np.float32)
    gq_sw = gq.reshape(-1, 2)[:, ::-1].reshape(-1)
    gk_sw = gk.reshape(-1, 2)[:, ::-1].reshape(-1)
    return (C * gq[None, :], Sx * gq_sw[None, :],
            C * gk[None, :], Sx * gk_sw[None, :])


def kernel(x, cache_k, cache_v, freqs_real, freqs_imag,
           wq, bq, wk, bk, wv, bv, wo, bo, gq, gk,
           f_frames, height, width, current_start, global_end_index,
           local_end_index):
    global LAST_RUNS
    LAST_RUNS = []

    x = np.asarray(x, np.float32)
    cache_k = np.asarray(cache_k, np.float32)
    cache_v = np.asarray(cache_v, np.float32)
    wq = np.asarray(wq, np.float32)
    wk = np.asarray(wk, np.float32)
    wv = np.asarray(wv, np.float32)
    wo = np.asarray(wo, np.float32)
    bo = np.asarray(bo, np.float32)
    f = int(f_frames)
    h = int(height)
    w = int(width)
    current_start = int(current_start)
    global_end_index = int(global_end_index)
    local_end_index = int(local_end_index)

    assert x.shape == (1, S, DIM)
    for b in (bq, bk, bv):
        assert not np.any(np.asarray(b)), "nonzero qkv bias unsupported"

    frame_seqlen = h * w
    start_frame = current_start // frame_seqlen

    Cq, Sq, Ck, Sk = _rope_tables(freqs_real, freqs_imag, f, h, w,
                                  start_frame, gq, gk)
    W_all = np.concatenate([wq, wk, wv], axis=1)
    w3 = np.ascontiguousarray(
        W_all.reshape(12, 128, 9, 512).transpose(2, 1, 0, 3)).astype(BF16)
    xT = x[0].T.astype(BF16)

    nc1 = _CACHED.get("l1")
    if nc1 is None:
        nc1 = _CACHED["l1"] = _build_launch1()

    in_maps1 = []
    for c in range(NCORES):
        r0, r1 = c * RPC, (c + 1) * RPC
        xt_c = np.ascontiguousarray(
            xT[:, r0:r1].reshape(12, 128, RPC).transpose(1, 0, 2))
        in_maps1.append({
            "xt": xt_c,
            "w3": w3,
            "cq": np.ascontiguousarray(Cq[r0:r1]).astype(BF16),
            "sq": np.ascontiguousarray(Sq[r0:r1]).astype(BF16),
            "ck": np.ascontiguousarray(Ck[r0:r1]).astype(BF16),
            "sk": np.ascontiguousarray(Sk[r0:r1]).astype(BF16),
        })
    res1 = bass_utils.run_bass_kernel_spmd(nc1, in_maps1,
                                           core_ids=list(range(NCORES)))
    LAST_RUNS.append(res1)
    qkv = np.concatenate([res1.results[c]["qkv"] for c in range(NCORES)],
                         axis=0)
    Q = qkv[:, :DIM]
    Knew = qkv[:, DIM:2 * DIM]
    Vnew = qkv[:, 2 * DIM:]

    old_rows, new_rows = _cache_plan(current_start, global_end_index,
                                     local_end_index, S, cache_k.shape[1],
                                     frame_seqlen)
    n_keys = len(old_rows) + len(new_rows)
    assert n_keys == CACHE, f"unexpected key count {n_keys}"

    K_eff = np.concatenate([
        cache_k[0, old_rows].reshape(len(old_rows), DIM).astype(BF16),
        Knew[new_rows],
    ], axis=0)
    V_eff = np.concatenate([
        cache_v[0, old_rows].reshape(len(old_rows), DIM).astype(BF16),
        Vnew[new_rows],
    ], axis=0)

    kt = np.ascontiguousarray(K_eff.T.reshape(NH, HD, CACHE))
    V_pad = np.zeros((NKC * 128, DIM), BF16)
    V_pad[:CACHE] = V_eff
    vt = np.ascontiguousarray(
        V_pad.reshape(NKC, 128, NH, HD).transpose(2, 1, 0, 3))
    w2 = np.ascontiguousarray(
        wo.reshape(12, 128, 3, 512).transpose(1, 0, 2, 3)).astype(BF16)
    bo2 = bo.reshape(1, DIM).astype(np.float32)

    nc2 = _CACHED.get("l2")
    if nc2 is None:
        nc2 = _CACHED["l2"] = _build_launch2()

    in_maps2 = []
    for c in range(NCORES):
        r0, r1 = c * RPC, (c + 1) * RPC
        qt_c = np.ascontiguousarray(
            Q[r0:r1].T.reshape(12, 128, RPC).transpose(1, 0, 2))
        in_maps2.append({
            "qt": qt_c,
            "kt": kt,
            "vt": vt,
            "w2": w2,
            "bo": bo2,
        })
    res2 = bass_utils.run_bass_kernel_spmd(nc2, in_maps2,
                                           core_ids=list(range(NCORES)))
    LAST_RUNS.append(res2)

    out = np.concatenate([res2.results[c]["outp"] for c in range(NCORES)],
                         axis=0)
    return out.reshape(1, S, DIM).astype(np.float32)

